# revision 27
# baseline (speedup 1.0000x reference)
"""Biaffine kernel for Trainium2, data-parallel over batch across 8 NeuronCores.

Math (reference):
  Ha = [H, 1]                                   # [B, N, d+1]
  out[b,x,y] = D[b,x,:] @ U @ Ha[b,y,:]  +  Ha[b,x,:]@W[:d+1]  +  D[b,y,:]@W[d+1:]

Decomposition used here (per batch b):
  U1 = U[:, :d]  (d x d),  u2 = U[:, d]
  G_b  = U1 @ H_b^T                             # [d, N]   (step 1, PE)
  S_b  = D_b @ G_b                              # [N, N]   (step 2, PE)
  rowvec[x] = D_b[x,:]@u2 + H_b[x,:]@W[:d]      # via skinny matmuls (vec3)
  colvec[y] = D_b[y,:]@W[d+1:] + W[d]
  out_b = S_b + rowvec x 1 + 1 x colvec         # folded into step 2 as a K=2 matmul

All matmuls in bf16 with fp32 PSUM accumulation. Host does layout/dtype prep only.
"""

import sys

for _p in ("/opt/trn_rl_repo", "/root/.axon_site/_ro/trn_rl_repo"):
    if _p not in sys.path:
        sys.path.append(_p)

import ml_dtypes
import numpy as np

B, N, DD = 64, 512, 1024
NCORES = 8
BPC = B // NCORES  # batches per core
P = 128
KC = DD // P  # 8 contraction chunks of 128
XC = N // P  # 4 output-row chunks of 128

BF16 = ml_dtypes.bfloat16

LAST_RESULT = None  # BassKernelResults of the most recent run (for test.py)


def _ensure_axon_ntff_hook():
    """Provide antenv.axon_hooks if the image lacks it, so trace=True works
    under axon. No-op when the real module exists or the .so is absent."""
    try:
        import antenv.axon_hooks  # noqa: F401
        return
    except ImportError:
        pass
    import contextlib
    import ctypes
    import os
    import types

    holder = {"hook": None, "built": False}

    def _build_hook():
        so_path = "/opt/axon/libaxon_pjrt.so"
        if not os.path.exists(so_path):
            return None
        lib = ctypes.CDLL(so_path)
        if not hasattr(lib, "axon_start_nrt_profile"):
            return None
        lib.axon_start_nrt_profile.argtypes = [
            ctypes.POINTER(ctypes.c_int64),
            ctypes.c_size_t,
        ]
        lib.axon_start_nrt_profile.restype = ctypes.c_int64
        lib.axon_stop_nrt_profile.argtypes = [ctypes.c_char_p]
        lib.axon_stop_nrt_profile.restype = ctypes.c_int64

        @contextlib.contextmanager
        def _hook(output_dir, device_ids):
            import jax

            jax.devices()
            if device_ids:
                ids = (ctypes.c_int64 * len(device_ids))(*device_ids)
                rc = lib.axon_start_nrt_profile(ids, len(device_ids))
            else:
                rc = lib.axon_start_nrt_profile(None, 0)
            if rc != 0:
                raise RuntimeError(f"axon_start_nrt_profile rc={rc}")
            try:
                yield
            finally:
                n = lib.axon_stop_nrt_profile(str(output_dir).encode())
                print(f"ntff profile: {n} file(s) -> {output_dir}")

        return _hook

    def set_axon_ntff_profile_hook(h):
        holder["hook"] = h
        holder["built"] = True

    def get_axon_ntff_profile_hook():
        if not holder["built"]:
            holder["hook"] = _build_hook()
            holder["built"] = True
        return holder["hook"]

    mod = types.ModuleType("antenv.axon_hooks")
    mod.set_axon_ntff_profile_hook = set_axon_ntff_profile_hook
    mod.get_axon_ntff_profile_hook = get_axon_ntff_profile_hook
    sys.modules["antenv.axon_hooks"] = mod
    try:
        import antenv

        antenv.axon_hooks = mod
    except ImportError:
        pass


def _build_bass(c_const: float):
    import concourse.mybir as mybir
    import concourse.tile as tile
    from concourse import bacc
    from contextlib import ExitStack

    bf = mybir.dt.bfloat16
    f32 = mybir.dt.float32

    nc = bacc.Bacc("TRN2")
    NP = BPC // 2  # batch pairs (ht is loaded/kept as pairs; matmuls stay N=512)
    dtr_h = nc.dram_tensor("dtr", [BPC, P, KC, N], bf, kind="ExternalInput")
    htr_h = nc.dram_tensor("htr", [NP, P, KC, 2 * N], bf, kind="ExternalInput")
    ujt_h = nc.dram_tensor("ujt", [P, KC, DD], bf, kind="ExternalInput")
    vpr_h = nc.dram_tensor("vpr", [P, KC, 2], bf, kind="ExternalInput")
    u2r_h = nc.dram_tensor("u2r", [P, KC], f32, kind="ExternalInput")
    out_h = nc.dram_tensor("out", [BPC, N, N], f32, kind="ExternalOutput")

    with tile.TileContext(nc) as tc, ExitStack() as ctx:
        const_pool = ctx.enter_context(tc.tile_pool(name="const", bufs=1))
        vp_s = const_pool.tile([P, KC, 2], bf, name="vp_s")
        nc.sync.dma_start(vp_s[:], vpr_h[:])
        u2_s = const_pool.tile([P, KC], f32, name="u2_s")
        nc.sync.dma_start(u2_s[:], u2r_h[:])
        ujt_s = const_pool.tile([P, KC, DD], bf, name="ujt_s")

        dpool = ctx.enter_context(tc.tile_pool(name="dpool", bufs=4))
        hpool = ctx.enter_context(tc.tile_pool(name="hpool", bufs=2))
        gpool = ctx.enter_context(tc.tile_pool(name="gpool", bufs=2))
        opool = ctx.enter_context(tc.tile_pool(name="opool", bufs=2))
        vecpool = ctx.enter_context(tc.tile_pool(name="vecpool", bufs=2))
        gps_pool = ctx.enter_context(tc.tile_pool(name="gps", bufs=3, space="PSUM"))
        sps_pool = ctx.enter_context(tc.tile_pool(name="sps", bufs=3, space="PSUM"))
        vps_pool = ctx.enter_context(tc.tile_pool(name="vps", bufs=1, space="PSUM"))

        ones_s = const_pool.tile([1, N], bf, name="ones_s")
        nc.vector.memset(ones_s[:], 1.0)

        state = {}  # per-batch tiles kept alive for the pipelined step 2

        def load_pair(p):
            ht2_t = hpool.tile([P, KC, 2 * N], bf, name=f"ht{p}", tag="ht")
            dts = []
            if p == 0:
                # first half of ht feeds the first matmuls; ujt chunks are only
                # needed after the 32 vec matmuls (~7us of cover)
                nc.sync.dma_start(ht2_t[:, :, 0:N], htr_h[p, :, :, 0:N])
                nc.sync.dma_start(ht2_t[:, :, N:2 * N], htr_h[p, :, :, N:2 * N])
                for b01 in range(2):
                    dt_t = dpool.tile([P, KC, N], bf, name=f"dt{2 * p + b01}", tag="dt")
                    nc.sync.dma_start(dt_t[:], dtr_h[2 * p + b01])
                    dts.append(dt_t)
                for jc in range(KC):
                    nc.sync.dma_start(ujt_s[:, jc, :], ujt_h[:, jc, :])
            else:
                nc.sync.dma_start(ht2_t[:], htr_h[p])
                for b01 in range(2):
                    dt_t = dpool.tile([P, KC, N], bf, name=f"dt{2 * p + b01}", tag="dt")
                    nc.sync.dma_start(dt_t[:], dtr_h[2 * p + b01])
                    dts.append(dt_t)
            return ht2_t, dts

        def vec_mms(b, ht_slice, dt_t):
            # rowvec_H = H.wh (D.u2 is folded into G); colvec = D.wd.
            # The two M=1 streams sit on distinct 32-col PE groups
            # (tile_position) so the interleaved matmul pairs run concurrently.
            ra = vps_pool.tile([1, N], f32, name=f"ra{b}", tag="ra")
            rb = vps_pool.tile([33, N], f32, name=f"rb{b}", tag="rb")
            for k in range(KC):
                nc.tensor.matmul(
                    ra[:], lhsT=vp_s[:, k, 0:1], rhs=ht_slice(k),
                    start=(k == 0), stop=(k == KC - 1), tile_position=(0, 0),
                )
                nc.tensor.matmul(
                    rb[32:33, :], lhsT=vp_s[:, k, 1:2], rhs=dt_t[:, k, :],
                    start=(k == 0), stop=(k == KC - 1), tile_position=(0, 32),
                )
            # lvec row0 = rowvec_H, row1 = ones ; rvec row0 = ones, row1 = colvec+c
            lvec = vecpool.tile([2, N], bf, name=f"lvec{b}", tag="lvec")
            nc.vector.tensor_copy(lvec[0:1, :], ra[:])
            nc.sync.dma_start(lvec[1:2, :], ones_s[0:1, :])
            rvec = vecpool.tile([2, N], bf, name=f"rvec{b}", tag="rvec")
            nc.vector.memset(rvec[0:1, :], 1.0)
            crow = vecpool.tile([1, N], bf, name=f"crow{b}", tag="crow")
            nc.vector.tensor_scalar_add(crow[:], rb[32:33, :], c_const)
            nc.sync.dma_start(rvec[1:2, :], crow[0:1, :])
            return lvec, rvec

        def step1_pair(p, ht2_t, dts):
            lr = []
            for b01 in range(2):
                lvec, rvec = vec_mms(
                    2 * p + b01,
                    lambda k, b01=b01: ht2_t[:, k, b01 * N:(b01 + 1) * N],
                    dts[b01],
                )
                lr.append((lvec, rvec))

            # step 1: G[i, y] = U1 @ H_b^T per batch (matmul free dim caps at 512);
            # the PSUM->SBUF cast adds u2[i] per partition, folding D.u2 into step 2
            g2_t = gpool.tile([P, KC, 2 * N], bf, name=f"g{p}", tag="g")
            for ic in range(KC):
                for b01 in range(2):
                    g_ps = gps_pool.tile([P, N], f32, name=f"gps{p}_{ic}_{b01}", tag="gps")
                    for jc in range(KC):
                        nc.tensor.matmul(
                            g_ps[:],
                            lhsT=ujt_s[:, jc, ic * P:(ic + 1) * P],
                            rhs=ht2_t[:, jc, b01 * N:(b01 + 1) * N],
                            start=(jc == 0), stop=(jc == KC - 1),
                        )
                    nc.vector.tensor_scalar_add(
                        g2_t[:, ic, b01 * N:(b01 + 1) * N], g_ps[:], u2_s[:, ic:ic + 1]
                    )

            for b01 in range(2):
                state[2 * p + b01] = (dts[b01], g2_t, b01, *lr[b01])

        def step2(b):
            dt_t, g2_t, b01, lvec, rvec = state.pop(b)
            o_t = opool.tile([P, XC, N], f32, name=f"o{b}", tag="o")
            for xc in range(XC):
                s_ps = sps_pool.tile([P, N], f32, name=f"sps{b}_{xc}", tag="sps")
                for ic in range(KC):
                    nc.tensor.matmul(
                        s_ps[:],
                        lhsT=dt_t[:, ic, xc * P:(xc + 1) * P],
                        rhs=g2_t[:, ic, b01 * N:(b01 + 1) * N],
                        start=(ic == 0), stop=False,
                    )
                # + rowvec[x] * 1 + 1 * colvec[y]  (one K=2 rank-2 matmul)
                nc.tensor.matmul(
                    s_ps[:],
                    lhsT=lvec[:, xc * P:(xc + 1) * P],
                    rhs=rvec[:, :],
                    start=False, stop=True,
                )
                nc.vector.tensor_copy(o_t[:, xc, :], s_ps[:])
                nc.sync.dma_start(out_h[b, xc * P:(xc + 1) * P, :], o_t[:, xc, :])

        # software-pipelined by one pair so PE never waits on the G copies
        for p in range(NP):
            ht2_t, dts = load_pair(p)
            step1_pair(p, ht2_t, dts)
            if p >= 1:
                step2(2 * p - 2)
                step2(2 * p - 1)
        step2(BPC - 2)
        step2(BPC - 1)

    nc.finalize()
    return nc


def kernel(D, H, U, W, _trace=False):
    global LAST_RESULT
    _ensure_axon_ntff_hook()
    from concourse.bass_utils import run_bass_kernel_spmd

    D = np.asarray(D, dtype=np.float32)
    H = np.asarray(H, dtype=np.float32)
    U = np.asarray(U, dtype=np.float32)
    W = np.asarray(W, dtype=np.float32)

    # ---- host-side layout / dtype prep (no math beyond the W[d] scalar) ----
    # dtr[b, p, c, x] = D[b, x, c*128+p]  (D^T, chunked along the contraction dim)
    DT = D.transpose(0, 2, 1).astype(BF16)  # [B, DD, N]
    dtr = np.ascontiguousarray(DT.reshape(B, KC, P, N).transpose(0, 2, 1, 3))
    HT = H.transpose(0, 2, 1).astype(BF16)
    htr = np.ascontiguousarray(HT.reshape(B, KC, P, N).transpose(0, 2, 1, 3))
    # paired layout for step-1 N=1024 streams: [pair, p, k, (b01, y)]
    htr = np.ascontiguousarray(
        htr.reshape(B // 2, 2, P, KC, N).transpose(0, 2, 3, 1, 4).reshape(B // 2, P, KC, 2 * N)
    )
    # ujt[p, jc, i] = U[i, jc*128+p]
    U1T = U[:, :DD].T  # [j, i]
    ujt = np.ascontiguousarray(U1T.reshape(KC, P, DD).transpose(1, 0, 2)).astype(BF16)
    # vpr[p, c, :] = (wh, wd)[c*128+p] ; u2r[p, c] = u2[c*128+p] (fp32, folded into G)
    vp = np.stack([W[:DD], W[DD + 1:]], axis=1)  # [DD, 2]
    vpr = np.ascontiguousarray(vp.reshape(KC, P, 2).transpose(1, 0, 2)).astype(BF16)
    u2r = np.ascontiguousarray(U[:, DD].reshape(KC, P).T).astype(np.float32)
    c_const = float(W[DD])

    nc = _build_bass(c_const)

    in_maps = []
    for c in range(NCORES):
        sl = slice(c * BPC, (c + 1) * BPC)
        slp = slice(c * (BPC // 2), (c + 1) * (BPC // 2))
        in_maps.append({
            "dtr": dtr[sl],
            "htr": htr[slp],
            "ujt": ujt,
            "vpr": vpr,
            "u2r": u2r,
        })

    try:
        res = run_bass_kernel_spmd(
            nc, in_maps, core_ids=list(range(NCORES)), trace=_trace,
        )
    except Exception:
        # transient device errors (e.g. NRT_EXEC_UNIT_UNRECOVERABLE) usually
        # clear on retry
        res = run_bass_kernel_spmd(
            nc, in_maps, core_ids=list(range(NCORES)), trace=_trace,
        )
    LAST_RESULT = res

    out = np.concatenate([r["out"] for r in res.results], axis=0)
    return np.ascontiguousarray(out.astype(np.float32))


if __name__ == "__main__":
    rng = np.random.default_rng(0)
    D = rng.standard_normal((B, N, DD), dtype=np.float32)
    H = rng.standard_normal((B, N, DD), dtype=np.float32)
    U = (rng.standard_normal((DD, DD + 1)) * 0.02).astype(np.float32)
    W = (rng.standard_normal((2 * DD + 1,)) * 0.02).astype(np.float32)
    out = kernel(D=D, H=H, U=U, W=W)
    print(out.shape, out.dtype)


# revision 28
# speedup vs baseline: 1.0428x; 1.0428x over previous
"""Biaffine kernel for Trainium2, data-parallel over batch across 8 NeuronCores.

Math (reference):
  Ha = [H, 1]                                   # [B, N, d+1]
  out[b,x,y] = D[b,x,:] @ U @ Ha[b,y,:]  +  Ha[b,x,:]@W[:d+1]  +  D[b,y,:]@W[d+1:]

Decomposition used here (per batch b):
  U1 = U[:, :d]  (d x d),  u2 = U[:, d]
  G_b  = U1 @ H_b^T                             # [d, N]   (step 1, PE)
  S_b  = D_b @ G_b                              # [N, N]   (step 2, PE)
  rowvec[x] = D_b[x,:]@u2 + H_b[x,:]@W[:d]      # via skinny matmuls (vec3)
  colvec[y] = D_b[y,:]@W[d+1:] + W[d]
  out_b = S_b + rowvec x 1 + 1 x colvec         # folded into step 2 as a K=2 matmul

All matmuls in bf16 with fp32 PSUM accumulation. Host does layout/dtype prep only.
"""

import sys

for _p in ("/opt/trn_rl_repo", "/root/.axon_site/_ro/trn_rl_repo"):
    if _p not in sys.path:
        sys.path.append(_p)

import ml_dtypes
import numpy as np

B, N, DD = 64, 512, 1024
NCORES = 8
BPC = B // NCORES  # batches per core
P = 128
KC = DD // P  # 8 contraction chunks of 128
XC = N // P  # 4 output-row chunks of 128

BF16 = ml_dtypes.bfloat16

LAST_RESULT = None  # BassKernelResults of the most recent run (for test.py)


def _ensure_axon_ntff_hook():
    """Provide antenv.axon_hooks if the image lacks it, so trace=True works
    under axon. No-op when the real module exists or the .so is absent."""
    try:
        import antenv.axon_hooks  # noqa: F401
        return
    except ImportError:
        pass
    import contextlib
    import ctypes
    import os
    import types

    holder = {"hook": None, "built": False}

    def _build_hook():
        so_path = "/opt/axon/libaxon_pjrt.so"
        if not os.path.exists(so_path):
            return None
        lib = ctypes.CDLL(so_path)
        if not hasattr(lib, "axon_start_nrt_profile"):
            return None
        lib.axon_start_nrt_profile.argtypes = [
            ctypes.POINTER(ctypes.c_int64),
            ctypes.c_size_t,
        ]
        lib.axon_start_nrt_profile.restype = ctypes.c_int64
        lib.axon_stop_nrt_profile.argtypes = [ctypes.c_char_p]
        lib.axon_stop_nrt_profile.restype = ctypes.c_int64

        @contextlib.contextmanager
        def _hook(output_dir, device_ids):
            import jax

            jax.devices()
            if device_ids:
                ids = (ctypes.c_int64 * len(device_ids))(*device_ids)
                rc = lib.axon_start_nrt_profile(ids, len(device_ids))
            else:
                rc = lib.axon_start_nrt_profile(None, 0)
            if rc != 0:
                raise RuntimeError(f"axon_start_nrt_profile rc={rc}")
            try:
                yield
            finally:
                n = lib.axon_stop_nrt_profile(str(output_dir).encode())
                print(f"ntff profile: {n} file(s) -> {output_dir}")

        return _hook

    def set_axon_ntff_profile_hook(h):
        holder["hook"] = h
        holder["built"] = True

    def get_axon_ntff_profile_hook():
        if not holder["built"]:
            holder["hook"] = _build_hook()
            holder["built"] = True
        return holder["hook"]

    mod = types.ModuleType("antenv.axon_hooks")
    mod.set_axon_ntff_profile_hook = set_axon_ntff_profile_hook
    mod.get_axon_ntff_profile_hook = get_axon_ntff_profile_hook
    sys.modules["antenv.axon_hooks"] = mod
    try:
        import antenv

        antenv.axon_hooks = mod
    except ImportError:
        pass


def _build_bass(c_const: float):
    import concourse.mybir as mybir
    import concourse.tile as tile
    from concourse import bacc
    from contextlib import ExitStack

    bf = mybir.dt.bfloat16
    f32 = mybir.dt.float32

    nc = bacc.Bacc("TRN2")
    NP = BPC // 2  # batch pairs (ht is loaded/kept as pairs; matmuls stay N=512)
    dtr_h = nc.dram_tensor("dtr", [BPC, P, KC, N], bf, kind="ExternalInput")
    htr_h = nc.dram_tensor("htr", [NP, P, KC, 2 * N], bf, kind="ExternalInput")
    ujt_h = nc.dram_tensor("ujt", [P, KC, DD], bf, kind="ExternalInput")
    vpr_h = nc.dram_tensor("vpr", [P, KC, 2], bf, kind="ExternalInput")
    u2r_h = nc.dram_tensor("u2r", [P, KC], f32, kind="ExternalInput")
    out_h = nc.dram_tensor("out", [BPC, N, N], f32, kind="ExternalOutput")

    with tile.TileContext(nc) as tc, ExitStack() as ctx:
        const_pool = ctx.enter_context(tc.tile_pool(name="const", bufs=1))
        vp_s = const_pool.tile([P, KC, 2], bf, name="vp_s")
        nc.sync.dma_start(vp_s[:], vpr_h[:])
        u2_s = const_pool.tile([P, KC], f32, name="u2_s")
        nc.sync.dma_start(u2_s[:], u2r_h[:])
        ujt_s = const_pool.tile([P, KC, DD], bf, name="ujt_s")

        dpool = ctx.enter_context(tc.tile_pool(name="dpool", bufs=4))
        hpool = ctx.enter_context(tc.tile_pool(name="hpool", bufs=2))
        gpool = ctx.enter_context(tc.tile_pool(name="gpool", bufs=2))
        opool = ctx.enter_context(tc.tile_pool(name="opool", bufs=2))
        vecpool = ctx.enter_context(tc.tile_pool(name="vecpool", bufs=2))
        gps_pool = ctx.enter_context(tc.tile_pool(name="gps", bufs=3, space="PSUM"))
        sps_pool = ctx.enter_context(tc.tile_pool(name="sps", bufs=3, space="PSUM"))
        vps_pool = ctx.enter_context(tc.tile_pool(name="vps", bufs=1, space="PSUM"))

        ones_s = const_pool.tile([1, N], bf, name="ones_s")
        nc.vector.memset(ones_s[:], 1.0)

        state = {}  # per-batch tiles kept alive for the pipelined step 2

        def load_pair(p):
            ht2_t = hpool.tile([P, KC, 2 * N], bf, name=f"ht{p}", tag="ht")
            dts = []
            if p == 0:
                # first half of ht feeds the first matmuls; ujt chunks are only
                # needed after the 32 vec matmuls (~7us of cover)
                nc.sync.dma_start(ht2_t[:, :, 0:N], htr_h[p, :, :, 0:N])
                nc.sync.dma_start(ht2_t[:, :, N:2 * N], htr_h[p, :, :, N:2 * N])
                for b01 in range(2):
                    dt_t = dpool.tile([P, KC, N], bf, name=f"dt{2 * p + b01}", tag="dt")
                    nc.sync.dma_start(dt_t[:], dtr_h[2 * p + b01])
                    dts.append(dt_t)
                for jc in range(KC):
                    nc.sync.dma_start(ujt_s[:, jc, :], ujt_h[:, jc, :])
            else:
                nc.sync.dma_start(ht2_t[:], htr_h[p])
                for b01 in range(2):
                    dt_t = dpool.tile([P, KC, N], bf, name=f"dt{2 * p + b01}", tag="dt")
                    nc.sync.dma_start(dt_t[:], dtr_h[2 * p + b01])
                    dts.append(dt_t)
            return ht2_t, dts

        def vec_mms_pair(p, ht2_t, dts):
            # Four M=1 contraction streams per pair -- rowvec_H = H.wh and
            # colvec = D.wd for both batches -- on distinct 32-col PE groups
            # (tile_position), interleaved per k so they run concurrently.
            # All land in one PSUM bank at quadrant-aligned partitions.
            v4 = vps_pool.tile([97, N], f32, name=f"v4_{p}", tag="v4")
            POS = (0, 32, 64, 96)  # (ra b0, rb b0, ra b1, rb b1)

            def rhs_for(s, k):
                if s == 0:
                    return ht2_t[:, k, 0:N]
                if s == 1:
                    return dts[0][:, k, :]
                if s == 2:
                    return ht2_t[:, k, N:2 * N]
                return dts[1][:, k, :]

            lcol = (0, 1, 0, 1)  # wh for ra streams, wd for rb streams
            for k in range(KC):
                for s in range(4):
                    nc.tensor.matmul(
                        v4[POS[s]:POS[s] + 1, :],
                        lhsT=vp_s[:, k, lcol[s]:lcol[s] + 1],
                        rhs=rhs_for(s, k),
                        start=(k == 0), stop=(k == KC - 1),
                        tile_position=(0, POS[s]),
                        skip_group_check=True,
                    )

            lr = []
            for b01 in range(2):
                b = 2 * p + b01
                # lvec row0 = rowvec_H, row1 = ones ; rvec row0 = ones, row1 = colvec+c
                lvec = vecpool.tile([2, N], bf, name=f"lvec{b}", tag="lvec")
                nc.vector.tensor_copy(lvec[0:1, :], v4[POS[2 * b01]:POS[2 * b01] + 1, :])
                nc.sync.dma_start(lvec[1:2, :], ones_s[0:1, :])
                rvec = vecpool.tile([2, N], bf, name=f"rvec{b}", tag="rvec")
                nc.vector.memset(rvec[0:1, :], 1.0)
                crow = vecpool.tile([1, N], bf, name=f"crow{b}", tag="crow")
                nc.vector.tensor_scalar_add(
                    crow[:], v4[POS[2 * b01 + 1]:POS[2 * b01 + 1] + 1, :], c_const
                )
                nc.sync.dma_start(rvec[1:2, :], crow[0:1, :])
                lr.append((lvec, rvec))
            return lr

        def step1_pair(p, ht2_t, dts):
            lr = vec_mms_pair(p, ht2_t, dts)

            # step 1: G[i, y] = U1 @ H_b^T per batch (matmul free dim caps at 512);
            # the PSUM->SBUF cast adds u2[i] per partition, folding D.u2 into step 2
            g2_t = gpool.tile([P, KC, 2 * N], bf, name=f"g{p}", tag="g")
            for ic in range(KC):
                for b01 in range(2):
                    g_ps = gps_pool.tile([P, N], f32, name=f"gps{p}_{ic}_{b01}", tag="gps")
                    for jc in range(KC):
                        nc.tensor.matmul(
                            g_ps[:],
                            lhsT=ujt_s[:, jc, ic * P:(ic + 1) * P],
                            rhs=ht2_t[:, jc, b01 * N:(b01 + 1) * N],
                            start=(jc == 0), stop=(jc == KC - 1),
                        )
                    nc.vector.tensor_scalar_add(
                        g2_t[:, ic, b01 * N:(b01 + 1) * N], g_ps[:], u2_s[:, ic:ic + 1]
                    )

            for b01 in range(2):
                state[2 * p + b01] = (dts[b01], g2_t, b01, *lr[b01])

        def step2(b):
            dt_t, g2_t, b01, lvec, rvec = state.pop(b)
            o_t = opool.tile([P, XC, N], f32, name=f"o{b}", tag="o")
            for xc in range(XC):
                s_ps = sps_pool.tile([P, N], f32, name=f"sps{b}_{xc}", tag="sps")
                for ic in range(KC):
                    nc.tensor.matmul(
                        s_ps[:],
                        lhsT=dt_t[:, ic, xc * P:(xc + 1) * P],
                        rhs=g2_t[:, ic, b01 * N:(b01 + 1) * N],
                        start=(ic == 0), stop=False,
                    )
                # + rowvec[x] * 1 + 1 * colvec[y]  (one K=2 rank-2 matmul)
                nc.tensor.matmul(
                    s_ps[:],
                    lhsT=lvec[:, xc * P:(xc + 1) * P],
                    rhs=rvec[:, :],
                    start=False, stop=True,
                )
                nc.vector.tensor_copy(o_t[:, xc, :], s_ps[:])
                nc.sync.dma_start(out_h[b, xc * P:(xc + 1) * P, :], o_t[:, xc, :])

        # software-pipelined by one pair so PE never waits on the G copies
        for p in range(NP):
            ht2_t, dts = load_pair(p)
            step1_pair(p, ht2_t, dts)
            if p >= 1:
                step2(2 * p - 2)
                step2(2 * p - 1)
        step2(BPC - 2)
        step2(BPC - 1)

    nc.finalize()
    return nc


def kernel(D, H, U, W, _trace=False):
    global LAST_RESULT
    _ensure_axon_ntff_hook()
    from concourse.bass_utils import run_bass_kernel_spmd

    D = np.asarray(D, dtype=np.float32)
    H = np.asarray(H, dtype=np.float32)
    U = np.asarray(U, dtype=np.float32)
    W = np.asarray(W, dtype=np.float32)

    # ---- host-side layout / dtype prep (no math beyond the W[d] scalar) ----
    # dtr[b, p, c, x] = D[b, x, c*128+p]  (D^T, chunked along the contraction dim)
    DT = D.transpose(0, 2, 1).astype(BF16)  # [B, DD, N]
    dtr = np.ascontiguousarray(DT.reshape(B, KC, P, N).transpose(0, 2, 1, 3))
    HT = H.transpose(0, 2, 1).astype(BF16)
    htr = np.ascontiguousarray(HT.reshape(B, KC, P, N).transpose(0, 2, 1, 3))
    # paired layout for step-1 N=1024 streams: [pair, p, k, (b01, y)]
    htr = np.ascontiguousarray(
        htr.reshape(B // 2, 2, P, KC, N).transpose(0, 2, 3, 1, 4).reshape(B // 2, P, KC, 2 * N)
    )
    # ujt[p, jc, i] = U[i, jc*128+p]
    U1T = U[:, :DD].T  # [j, i]
    ujt = np.ascontiguousarray(U1T.reshape(KC, P, DD).transpose(1, 0, 2)).astype(BF16)
    # vpr[p, c, :] = (wh, wd)[c*128+p] ; u2r[p, c] = u2[c*128+p] (fp32, folded into G)
    vp = np.stack([W[:DD], W[DD + 1:]], axis=1)  # [DD, 2]
    vpr = np.ascontiguousarray(vp.reshape(KC, P, 2).transpose(1, 0, 2)).astype(BF16)
    u2r = np.ascontiguousarray(U[:, DD].reshape(KC, P).T).astype(np.float32)
    c_const = float(W[DD])

    nc = _build_bass(c_const)

    in_maps = []
    for c in range(NCORES):
        sl = slice(c * BPC, (c + 1) * BPC)
        slp = slice(c * (BPC // 2), (c + 1) * (BPC // 2))
        in_maps.append({
            "dtr": dtr[sl],
            "htr": htr[slp],
            "ujt": ujt,
            "vpr": vpr,
            "u2r": u2r,
        })

    try:
        res = run_bass_kernel_spmd(
            nc, in_maps, core_ids=list(range(NCORES)), trace=_trace,
        )
    except Exception:
        # transient device errors (e.g. NRT_EXEC_UNIT_UNRECOVERABLE) usually
        # clear on retry
        res = run_bass_kernel_spmd(
            nc, in_maps, core_ids=list(range(NCORES)), trace=_trace,
        )
    LAST_RESULT = res

    out = np.concatenate([r["out"] for r in res.results], axis=0)
    return np.ascontiguousarray(out.astype(np.float32))


if __name__ == "__main__":
    rng = np.random.default_rng(0)
    D = rng.standard_normal((B, N, DD), dtype=np.float32)
    H = rng.standard_normal((B, N, DD), dtype=np.float32)
    U = (rng.standard_normal((DD, DD + 1)) * 0.02).astype(np.float32)
    W = (rng.standard_normal((2 * DD + 1,)) * 0.02).astype(np.float32)
    out = kernel(D=D, H=H, U=U, W=W)
    print(out.shape, out.dtype)
